# revision 1
# baseline (speedup 1.0000x reference)
import sys
if '/opt/trn_rl_repo' not in sys.path:
    sys.path.insert(0, '/opt/trn_rl_repo')
import numpy as np

import concourse.bass as bass
import concourse.bacc as bacc
import concourse.tile as tile
from concourse import mybir
from concourse import bass_utils

f32 = mybir.dt.float32
f32r = mybir.dt.float32r
FX = mybir.ActivationFunctionType
ALU = mybir.AluOpType
AX = mybir.AxisListType

B, D, H, DH = 256, 256, 8, 32
NCORES = 8
BC = B // NCORES          # 32 batches per core
LC = 1024                 # self-attn KV cache length
NA = 2048                 # cross-attn key count
KT_S = LC // 128          # 8 key tiles (self)
KT_A = NA // 128          # 16 key tiles (cross)
SCALE = 1.0 / float(np.sqrt(DH))
EPS = 1e-5

WNAMES = ['wq_s', 'wk_s', 'wv_s', 'w0_s', 'wq_a', 'w0_a', 'w1', 'w2']
BNAMES = ['bq_s', 'bk_s', 'bv_s', 'b0_s', 'bq_a', 'b0_a', 'b1', 'b2']
LNAMES = ['ln1_g', 'ln1_b', 'ln2_g', 'ln2_b', 'ln3_g', 'ln3_b']


def _build():
    nc = bacc.Bacc()
    dr = {}
    dr['h_t'] = nc.dram_tensor('h_t', [BC, 1, D], f32, kind='ExternalInput')
    dr['K_att'] = nc.dram_tensor('K_att', [BC, NA, D], f32, kind='ExternalInput')
    dr['V_att'] = nc.dram_tensor('V_att', [BC, NA, D], f32r, kind='ExternalInput')
    dr['K_cache'] = nc.dram_tensor('K_cache', [BC, LC, D], f32, kind='ExternalInput')
    dr['V_cache'] = nc.dram_tensor('V_cache', [BC, LC, D], f32r, kind='ExternalInput')
    dr['notmT'] = nc.dram_tensor('notmT', [128, KT_A, BC], f32, kind='ExternalInput')
    dr['ident'] = nc.dram_tensor('ident', [128, 128], f32, kind='ExternalInput')
    dr['ones'] = nc.dram_tensor('ones', [128, 1], f32r, kind='ExternalInput')
    dr['zeros'] = nc.dram_tensor('zeros', [128, 2 * H * BC], f32r, kind='ExternalInput')
    for n in WNAMES:
        dr[n] = nc.dram_tensor(n, [D, D], f32r, kind='ExternalInput')
    for n in BNAMES + LNAMES:
        dr[n] = nc.dram_tensor(n, [D], f32, kind='ExternalInput')
    out = nc.dram_tensor('out', [BC, D], f32, kind='ExternalOutput')

    with tile.TileContext(nc) as tc:
        _emit(nc, tc, dr, out)
    nc.compile()
    return nc


def _emit(nc, tc, dr, out_dram):
    import contextlib
    ctx = contextlib.ExitStack()
    with ctx:
        const = ctx.enter_context(tc.tile_pool(name='const', bufs=1))
        keys_p = ctx.enter_context(tc.tile_pool(name='keys', bufs=3))
        vals_p = ctx.enter_context(tc.tile_pool(name='vals', bufs=3))
        ktl_p = ctx.enter_context(tc.tile_pool(name='ktl', bufs=2))
        kth_p = ctx.enter_context(tc.tile_pool(name='kth', bufs=2))
        wb_p = ctx.enter_context(tc.tile_pool(name='wb', bufs=2))
        wt_p = ctx.enter_context(tc.tile_pool(name='wt', bufs=2))
        sm_p = ctx.enter_context(tc.tile_pool(name='sm', bufs=4))
        tr_ps = ctx.enter_context(tc.tile_pool(name='trps', bufs=2, space='PSUM'))
        sc_ps = ctx.enter_context(tc.tile_pool(name='scps', bufs=2, space='PSUM'))
        at_ps = ctx.enter_context(tc.tile_pool(name='atps', bufs=2, space='PSUM'))
        ln_ps = ctx.enter_context(tc.tile_pool(name='lnps', bufs=1, space='PSUM'))
        gb_ps = ctx.enter_context(tc.tile_pool(name='gbps', bufs=1, space='PSUM'))

        garb = gb_ps.tile([1, 1], f32, tag='garb')
        last_act = [None]

        def pe_absorb(*aps):
            # PE matmul/transpose (fp32/fp32r self-loading weights) can carry only ONE
            # sem wait in its LW slot. Before a matmul whose deps span several procs,
            # emit 1x1 self-matmuls so the PE observes those sems here instead.
            for a in aps:
                if a is None:
                    continue
                e = a[tuple(slice(0, 1) for _ in range(len(a.shape)))]
                if e.dtype == f32r:
                    e = e.bitcast(f32)
                nc.tensor.matmul(garb[:, :], e, e, start=True, stop=True,
                                 skip_group_check=True)

        # ---------- persistent loads ----------
        ident = const.tile([128, 128], f32, tag='ident')
        nc.sync.dma_start(out=ident, in_=dr['ident'][:, :])
        pe_absorb(ident)
        ones = const.tile([128, 1], f32r, tag='ones')
        nc.sync.dma_start(out=ones, in_=dr['ones'][:, :])
        epst = const.tile([BC, 1], f32, tag='epst')
        nc.vector.memset(epst, EPS)

        wsb = {}
        for n in WNAMES:
            wsb[n] = const.tile([128, 2, D], f32r, tag='w_' + n, name='w_' + n)
            nc.sync.dma_start(out=wsb[n], in_=dr[n][:, :].rearrange('(t p) j -> p t j', p=128))
        vsb = {}
        for n in BNAMES + LNAMES:
            vsb[n] = const.tile([BC, D], f32, tag='v_' + n, name='v_' + n)
            nc.gpsimd.dma_start(out=vsb[n], in_=dr[n][:].unsqueeze(0).to_broadcast([BC, D]))

        notmT = const.tile([128, KT_A, BC], f32, tag='notmT')
        nc.sync.dma_start(out=notmT, in_=dr['notmT'][:, :, :])

        ht = const.tile([BC, D], f32, tag='ht')
        nc.sync.dma_start(out=ht, in_=dr['h_t'][:, 0, :])
        pe_absorb(ht)

        # ---------- helpers ----------
        def transpose_128(dst, src, cols):
            # src [rows<=128, cols<=128] SBUF f32 -> dst [cols, rows] via PE transpose
            rows = src.shape[0]
            ps = tr_ps.tile([128, 128], f32, tag='trps')
            nc.tensor.transpose(ps[0:cols, 0:rows], src, ident[0:rows, 0:rows])
            nc.vector.tensor_copy(out=dst, in_=ps[0:cols, 0:rows])

        def make_T(src_f32, tagname):
            # src [BC, D] -> [128, 2, BC] f32r transposed halves
            dstT = const.tile([128, 2, BC], f32r, tag=tagname, name=tagname)
            for t in range(2):
                transpose_128(dstT[:, t, :], src_f32[:, 128 * t:128 * (t + 1)], 128)
            return dstT

        def linear_psum(srcT_list, wname):
            # sum_t sum_s srcT.T @ W  -> psum [BC, D]
            ps = ln_ps.tile([BC, D], f32, tag='lnps')
            pe_absorb(wsb[wname])
            n_mm = 2 * len(srcT_list)
            i = 0
            for srcT in srcT_list:
                for t in range(2):
                    nc.tensor.matmul(ps[:, :], srcT[:, t, :], wsb[wname][:, t, :],
                                     start=(i == 0), stop=(i == n_mm - 1))
                    i += 1
            return ps

        def layernorm(dst, src, gname, bname, tagp):
            stats = const.tile([BC, 6], f32, tag=tagp + '_st', name=tagp + '_st')
            nc.vector.bn_stats(out=stats, in_=src)
            mv = const.tile([BC, 2], f32, tag=tagp + '_mv', name=tagp + '_mv')
            nc.vector.bn_aggr(out=mv, in_=stats)
            sd = const.tile([BC, 1], f32, tag=tagp + '_sd', name=tagp + '_sd')
            nc.scalar.activation(out=sd, in_=mv[:, 1:2], func=FX.Sqrt,
                                 bias=epst[:, :], scale=1.0)
            rstd = const.tile([BC, 1], f32, tag=tagp + '_rs', name=tagp + '_rs')
            nc.vector.reciprocal(out=rstd, in_=sd)
            nc.vector.tensor_scalar(out=dst, in0=src, scalar1=mv[:, 0:1], scalar2=rstd,
                                    op0=ALU.subtract, op1=ALU.mult)
            nc.vector.tensor_mul(dst, dst, vsb[gname])
            nc.vector.tensor_add(dst, dst, vsb[bname])

        def build_qblk(qsrc_f32, tagp):
            qT = make_T(qsrc_f32, tagp + '_qT')
            qb = const.tile([128, 2, H, BC], f32r, tag=tagp + '_qb', name=tagp + '_qb')
            nc.sync.dma_start(out=qb, in_=dr['zeros'][:, :].rearrange('p (t h b) -> p t h b', t=2, h=H))
            pe_absorb(qb)
            for t in range(2):
                for hh in range(4):
                    h = 4 * t + hh
                    nc.vector.tensor_copy(out=qb[32 * hh:32 * (hh + 1), t, h, :],
                                          in_=qT[32 * hh:32 * (hh + 1), t, :])
            return qb

        # ---------- qkv for self-attn ----------
        htT = make_T(ht, 'htT')
        qkv = {}
        for nm, wn, bn in (('q', 'wq_s', 'bq_s'), ('k', 'wk_s', 'bk_s'), ('v', 'wv_s', 'bv_s')):
            ps = linear_psum([htT], wn)
            qkv[nm] = const.tile([BC, D], f32, tag='qkv_' + nm, name='qkv_' + nm)
            nc.vector.tensor_add(qkv[nm], ps, vsb[bn])

        qblk_s = build_qblk(qkv['q'], 'self')

        # new-key (appended k/v) terms, all-batch
        qk = const.tile([BC, D], f32, tag='qk')
        nc.vector.tensor_mul(qk, qkv['q'], qkv['k'])
        s_new = const.tile([BC, H], f32, tag='s_new')
        nc.vector.reduce_sum(out=s_new, in_=qk.rearrange('p (g s) -> p g s', g=H), axis=AX.X)
        w_new = const.tile([BC, H], f32, tag='w_new')
        nc.scalar.activation(out=w_new, in_=s_new, func=FX.Exp, scale=SCALE)
        w_newT = const.tile([H, BC], f32, tag='w_newT')
        pe_absorb(w_new)
        transpose_128(w_newT, w_new, H)

        invmix = const.tile([H, BC], f32, tag='invmix')

        # ---------- attention inner loop ----------
        def attention(qblk, n_tiles, K_dram, V_dram, attT_dst, masked, inv_store):
            for b in range(BC):
                kc = keys_p.tile([128, KT_A, D], f32, tag='keys')
                nc.sync.dma_start(out=kc[:, 0:n_tiles, :],
                                  in_=K_dram[b].rearrange('(t p) d -> p t d', p=128))
                vc = vals_p.tile([128, KT_A, D + 4], f32r, tag='vals')
                nc.sync.dma_start(out=vc[:, 0:n_tiles, 0:D],
                                  in_=V_dram[b].rearrange('(t p) d -> p t d', p=128))
                nc.vector.tensor_copy(out=vc[:, 0:n_tiles, D:D + 4],
                                      in_=ones.unsqueeze(1).broadcast_to([128, n_tiles, 4]))
                pe_absorb(kc, vc)
                ktl = ktl_p.tile([128, KT_A, 128], f32r, tag='ktl')
                kth = kth_p.tile([128, KT_A, 128], f32r, tag='kth')
                for t in range(n_tiles):
                    ps1 = tr_ps.tile([128, 128], f32, tag='trps')
                    nc.tensor.transpose(ps1[:, :], kc[:, t, 0:128], ident)
                    nc.vector.tensor_copy(out=ktl[:, t, :], in_=ps1)
                    ps2 = tr_ps.tile([128, 128], f32, tag='trps')
                    nc.tensor.transpose(ps2[:, :], kc[:, t, 128:256], ident)
                    nc.vector.tensor_copy(out=kth[:, t, :], in_=ps2)
                wb = wb_p.tile([H, KT_A * 128], f32, tag='wb')
                for c in range(n_tiles // 4):
                    ssp = sc_ps.tile([H, 512], f32, tag='scps')
                    if last_act[0] is not None:
                        pe_absorb(last_act[0])
                    nc.tensor.matmul(ssp[:, :], qblk[:, 0, :, b], ktl[:, 4 * c:4 * (c + 1), :],
                                     start=True, stop=False)
                    nc.tensor.matmul(ssp[:, :], qblk[:, 1, :, b], kth[:, 4 * c:4 * (c + 1), :],
                                     start=False, stop=True)
                    nc.scalar.activation(out=wb[:, 512 * c:512 * (c + 1)], in_=ssp,
                                         func=FX.Exp, scale=SCALE)
                    last_act[0] = wb[:, 512 * c:512 * (c + 1)]
                atp = at_ps.tile([H, D + 4], f32, tag='atps')
                wtt = wt_p.tile([128, KT_A, H], f32r, tag='wt')
                for t in range(n_tiles):
                    if t % 4 == 0:
                        pe_absorb(wb[:, 512 * (t // 4):512 * (t // 4) + 1])
                    pw = tr_ps.tile([128, 128], f32, tag='trps')
                    nc.tensor.transpose(pw[0:128, 0:H], wb[:, 128 * t:128 * (t + 1)],
                                        ident[0:H, 0:H])
                    if masked:
                        nc.vector.tensor_scalar_mul(out=wtt[:, t, :], in0=pw[:, 0:H],
                                                    scalar1=notmT[:, t, b:b + 1])
                    else:
                        nc.vector.tensor_copy(out=wtt[:, t, :], in_=pw[:, 0:H])
                    nc.tensor.matmul(atp[:, :], wtt[:, t, :], vc[:, t, :],
                                     start=(t == 0), stop=(t == n_tiles - 1),
                                     skip_group_check=True)
                # denominator -> inverse
                dn = sm_p.tile([H, 1], f32, tag='dn')
                if inv_store is not None:
                    nc.vector.tensor_add(dn, atp[:, D:D + 1], w_newT[:, b:b + 1])
                else:
                    nc.vector.tensor_copy(out=dn, in_=atp[:, D:D + 1])
                iv = sm_p.tile([H, 1], f32, tag='iv')
                nc.vector.reciprocal(out=iv, in_=dn)
                if inv_store is not None:
                    nc.vector.tensor_copy(out=inv_store[:, b:b + 1], in_=iv)
                # scaled mixed attention, then un-mix via transpose + 32-aligned copies
                attm = sm_p.tile([H, D], f32, tag='attm')
                nc.vector.tensor_scalar_mul(out=attm, in0=atp[:, 0:D], scalar1=iv)
                for t in range(2):
                    pa = tr_ps.tile([128, 128], f32, tag='trps')
                    nc.tensor.transpose(pa[0:128, 0:H], attm[:, 128 * t:128 * (t + 1)],
                                        ident[0:H, 0:H])
                    for k in range(4):
                        h = 4 * t + k
                        nc.vector.tensor_copy(out=attT_dst[32 * k:32 * (k + 1), t, b:b + 1],
                                              in_=pa[32 * k:32 * (k + 1), h:h + 1])

        # ---------- self attention ----------
        attT_s = const.tile([128, 2, BC], f32r, tag='attT_s')
        attention(qblk_s, KT_S, dr['K_cache'], dr['V_cache'], attT_s, False, invmix)

        # new-key numerator: nv = v * w_new * inv  (batch layout), then transpose
        invb = const.tile([BC, H], f32, tag='invb')
        transpose_128(invb, invmix, BC)
        nv = const.tile([BC, D], f32, tag='nv')
        nc.vector.tensor_tensor(out=nv.rearrange('p (g s) -> p g s', g=H),
                                in0=qkv['v'].rearrange('p (g s) -> p g s', g=H),
                                in1=w_new.unsqueeze(2).broadcast_to([BC, H, DH]),
                                op=ALU.mult)
        nc.vector.tensor_tensor(out=nv.rearrange('p (g s) -> p g s', g=H),
                                in0=nv.rearrange('p (g s) -> p g s', g=H),
                                in1=invb.unsqueeze(2).broadcast_to([BC, H, DH]),
                                op=ALU.mult)
        nvT = make_T(nv, 'nvT')

        # h1 = LN1(ht + att_self @ w0_s + b0_s)
        ps = linear_psum([attT_s, nvT], 'w0_s')
        h1p = const.tile([BC, D], f32, tag='h1p')
        nc.vector.tensor_add(h1p, ps, vsb['b0_s'])
        nc.vector.tensor_add(h1p, h1p, ht)
        h1 = const.tile([BC, D], f32, tag='h1')
        layernorm(h1, h1p, 'ln1_g', 'ln1_b', 'ln1')

        # ---------- cross attention ----------
        h1T = make_T(h1, 'h1T')
        psq = linear_psum([h1T], 'wq_a')
        qa = const.tile([BC, D], f32, tag='qa')
        nc.vector.tensor_add(qa, psq, vsb['bq_a'])
        qblk_a = build_qblk(qa, 'cross')

        attT_a = const.tile([128, 2, BC], f32r, tag='attT_a')
        attention(qblk_a, KT_A, dr['K_att'], dr['V_att'], attT_a, True, None)

        # h2 = LN2(h1 + att_cross @ w0_a + b0_a)
        ps2 = linear_psum([attT_a], 'w0_a')
        h2p = const.tile([BC, D], f32, tag='h2p')
        nc.vector.tensor_add(h2p, ps2, vsb['b0_a'])
        nc.vector.tensor_add(h2p, h2p, h1)
        h2 = const.tile([BC, D], f32, tag='h2')
        layernorm(h2, h2p, 'ln2_g', 'ln2_b', 'ln2')

        # ---------- MLP ----------
        h2T = make_T(h2, 'h2T')
        psm = linear_psum([h2T], 'w1')
        m1 = const.tile([BC, D], f32, tag='m1')
        nc.vector.tensor_add(m1, psm, vsb['b1'])
        m1r = const.tile([BC, D], f32, tag='m1r')
        nc.scalar.activation(out=m1r, in_=m1, func=FX.Relu, scale=1.0)
        pe_absorb(m1r)
        m1T = make_T(m1r, 'm1T')
        psm2 = linear_psum([m1T], 'w2')
        h3p = const.tile([BC, D], f32, tag='h3p')
        nc.vector.tensor_add(h3p, psm2, vsb['b2'])
        nc.vector.tensor_add(h3p, h3p, h2)
        outt = const.tile([BC, D], f32, tag='outt')
        layernorm(outt, h3p, 'ln3_g', 'ln3_b', 'ln3')
        nc.sync.dma_start(out=out_dram[:, :], in_=outt)


_CACHE = {}


def _get_nc():
    if 'nc' not in _CACHE:
        _CACHE['nc'] = _build()
    return _CACHE['nc']


def _make_in_maps(inputs):
    np_in = {k: np.ascontiguousarray(np.asarray(v)) for k, v in inputs.items()}
    ident = np.eye(128, dtype=np.float32)
    ones = np.ones((128, 1), dtype=np.float32)
    zeros = np.zeros((128, 2 * H * BC), dtype=np.float32)
    in_maps = []
    for c in range(NCORES):
        sl = slice(c * BC, (c + 1) * BC)
        m = np_in['mask'][sl].astype(np.float32)          # [BC, NA], True = masked
        notm = (1.0 - m).reshape(BC, KT_A, 128).transpose(2, 1, 0).copy()  # [128, KT_A, BC]
        im = {
            'h_t': np_in['h_t'][sl],
            'K_att': np_in['K_att'][sl],
            'V_att': np_in['V_att'][sl],
            'K_cache': np_in['K_cache'][sl],
            'V_cache': np_in['V_cache'][sl],
            'notmT': notm,
            'ident': ident,
            'ones': ones,
            'zeros': zeros,
        }
        for n in WNAMES + BNAMES + LNAMES:
            im[n] = np_in[n]
        in_maps.append(im)
    return in_maps


def run_on_device(inputs):
    nc = _get_nc()
    in_maps = _make_in_maps(inputs)
    res = bass_utils.run_bass_kernel_spmd(nc, in_maps, core_ids=list(range(NCORES)),
                                          trace=False)
    outs = [res.results[c]['out'] for c in range(NCORES)]
    return np.concatenate(outs, axis=0).astype(np.float32)


def kernel(**inputs):
    return run_on_device(inputs)



# revision 7
# speedup vs baseline: 1.0420x; 1.0420x over previous
import sys
if '/opt/trn_rl_repo' not in sys.path:
    sys.path.insert(0, '/opt/trn_rl_repo')
import numpy as np

import concourse.bass as bass
import concourse.bacc as bacc
import concourse.tile as tile
from concourse import mybir
from concourse import bass_utils

f32 = mybir.dt.float32
f32r = mybir.dt.float32r
bf16 = mybir.dt.bfloat16
FX = mybir.ActivationFunctionType
ALU = mybir.AluOpType
AX = mybir.AxisListType

B, D, H, DH = 256, 256, 8, 32
NCORES = 8
BC = B // NCORES          # 32 batches per core
LC = 1024                 # self-attn KV cache length
NA = 2048                 # cross-attn key count
KT_S = LC // 128          # 8 key tiles (self)
KT_A = NA // 128          # 16 key tiles (cross)
SCALE = 1.0 / float(np.sqrt(DH))
EPS = 1e-5

KDT = bf16                # dtype for K^T tiles + q blocks (scores path)
VDT = bf16                # dtype for V tiles + softmax weights (mix path)
KDT_NP = mybir.dt.np(KDT)
VDT_NP = mybir.dt.np(VDT)

WNAMES = ['wq_s', 'wk_s', 'wv_s', 'w0_s', 'wq_a', 'w0_a', 'w1', 'w2']
BNAMES = ['bq_s', 'bk_s', 'bv_s', 'b0_s', 'bq_a', 'b0_a', 'b1', 'b2']
LNAMES = ['ln1_g', 'ln1_b', 'ln2_g', 'ln2_b', 'ln3_g', 'ln3_b']


def _build():
    nc = bacc.Bacc()
    dr = {}
    dr['h_t'] = nc.dram_tensor('h_t', [BC, 1, D], f32, kind='ExternalInput')
    # K^T tiles: [b, p(d%128), s(d//128), t, j(key%128)]
    dr['KaT'] = nc.dram_tensor('KaT', [BC, 128, 2, KT_A, 128], KDT, kind='ExternalInput')
    dr['KcT'] = nc.dram_tensor('KcT', [BC, 128, 2, KT_S, 128], KDT, kind='ExternalInput')
    # V tiles: [b, p(key%128), t, c(d//128), j(d%128)]
    dr['Va'] = nc.dram_tensor('Va', [BC, 128, KT_A, 2, 128], VDT, kind='ExternalInput')
    dr['Vc'] = nc.dram_tensor('Vc', [BC, 128, KT_S, 2, 128], VDT, kind='ExternalInput')
    dr['notmT'] = nc.dram_tensor('notmT', [128, KT_A, BC], f32, kind='ExternalInput')
    dr['ident'] = nc.dram_tensor('ident', [128, 128], f32, kind='ExternalInput')
    dr['onesrow'] = nc.dram_tensor('onesrow', [1, 128], f32r, kind='ExternalInput')
    for n in WNAMES:
        dr[n] = nc.dram_tensor(n, [D, D], f32r, kind='ExternalInput')
    for n in BNAMES + LNAMES:
        dr[n] = nc.dram_tensor(n, [D], f32, kind='ExternalInput')
    out = nc.dram_tensor('out', [BC, D], f32, kind='ExternalOutput')

    with tile.TileContext(nc) as tc:
        _emit(nc, tc, dr, out)
    nc.compile()
    return nc


def _emit(nc, tc, dr, out_dram):
    import contextlib
    ctx = contextlib.ExitStack()
    with ctx:
        const = ctx.enter_context(tc.tile_pool(name='const', bufs=1))
        kta_p = ctx.enter_context(tc.tile_pool(name='kta', bufs=3))
        vta_p = ctx.enter_context(tc.tile_pool(name='vta', bufs=3))
        kts_p = ctx.enter_context(tc.tile_pool(name='kts', bufs=3))
        vts_p = ctx.enter_context(tc.tile_pool(name='vts', bufs=3))
        wsb_p = ctx.enter_context(tc.tile_pool(name='wsb', bufs=3))
        sc_ps = ctx.enter_context(tc.tile_pool(name='scps', bufs=2, space='PSUM'))
        at_ps = ctx.enter_context(tc.tile_pool(name='atps', bufs=2, space='PSUM'))
        tr_ps = ctx.enter_context(tc.tile_pool(name='trps', bufs=1, space='PSUM'))
        rp_ps = ctx.enter_context(tc.tile_pool(name='rpps', bufs=1, space='PSUM'))
        ln_ps = ctx.enter_context(tc.tile_pool(name='lnps', bufs=1, space='PSUM'))
        gb_ps = ctx.enter_context(tc.tile_pool(name='gbps', bufs=1, space='PSUM'))

        garb = gb_ps.tile([1, 1], f32, tag='garb')

        def pe_absorb(*aps):
            # PE matmul (self-loading weights) can carry only ONE sem wait in
            # its LW slot. Before a matmul whose deps span several producers,
            # emit 1x1 self-matmuls so the PE observes those sems here.
            for a in aps:
                if a is None:
                    continue
                e = a[tuple(slice(0, 1) for _ in range(len(a.shape)))]
                if e.dtype == f32r:
                    e = e.bitcast(f32)
                nc.tensor.matmul(garb[:, :], e, e, start=True, stop=True,
                                 skip_group_check=True)

        # ---------- persistent loads / consts ----------
        ident = const.tile([128, 128], f32, tag='ident')
        nc.sync.dma_start(out=ident, in_=dr['ident'][:, :])
        pe_absorb(ident)
        epst = const.tile([BC, 1], f32, tag='epst')
        nc.vector.memset(epst, EPS)
        ones128 = const.tile([128, 1], VDT, tag='ones128')
        nc.vector.memset(ones128, 1.0)
        onescol = const.tile([1, 128], f32r, tag='onescol')
        nc.sync.dma_start(out=onescol, in_=dr['onesrow'][:, :])

        wsb = {}
        for n in WNAMES:
            wsb[n] = const.tile([128, 2, D], f32r, tag='w_' + n, name='w_' + n)
            nc.sync.dma_start(out=wsb[n], in_=dr[n][:, :].rearrange('(t p) j -> p t j', p=128))
        vsb = {}
        for n in BNAMES + LNAMES:
            vsb[n] = const.tile([BC, D], f32, tag='v_' + n, name='v_' + n)
            nc.gpsimd.dma_start(out=vsb[n], in_=dr[n][:].unsqueeze(0).to_broadcast([BC, D]))

        notmT = const.tile([128, KT_A, BC], f32, tag='notmT')
        nc.sync.dma_start(out=notmT, in_=dr['notmT'][:, :, :])

        ht = const.tile([BC, D], f32, tag='ht')
        nc.sync.dma_start(out=ht, in_=dr['h_t'][:, 0, :])
        pe_absorb(ht)

        # ---------- helpers ----------
        def transpose_128(dst, src, cols):
            rows = src.shape[0]
            ps = tr_ps.tile([128, 128], f32, tag='trps')
            nc.tensor.transpose(ps[0:cols, 0:rows], src, ident[0:rows, 0:rows])
            nc.vector.tensor_copy(out=dst, in_=ps[0:cols, 0:rows])

        def make_T(src_f32, tagname):
            dstT = const.tile([128, 2, BC], f32r, tag=tagname, name=tagname)
            for t in range(2):
                transpose_128(dstT[:, t, :], src_f32[:, 128 * t:128 * (t + 1)], 128)
            return dstT

        def linear_psum(srcT_list, wname):
            ps = ln_ps.tile([BC, D], f32, tag='lnps')
            pe_absorb(wsb[wname])
            n_mm = 2 * len(srcT_list)
            i = 0
            for srcT in srcT_list:
                for t in range(2):
                    nc.tensor.matmul(ps[:, :], srcT[:, t, :], wsb[wname][:, t, :],
                                     start=(i == 0), stop=(i == n_mm - 1))
                    i += 1
            return ps

        def layernorm(dst, src, gname, bname, tagp):
            stats = const.tile([BC, 6], f32, tag=tagp + '_st', name=tagp + '_st')
            nc.vector.bn_stats(out=stats, in_=src)
            mv = const.tile([BC, 2], f32, tag=tagp + '_mv', name=tagp + '_mv')
            nc.vector.bn_aggr(out=mv, in_=stats)
            sd = const.tile([BC, 1], f32, tag=tagp + '_sd', name=tagp + '_sd')
            nc.scalar.activation(out=sd, in_=mv[:, 1:2], func=FX.Sqrt,
                                 bias=epst[:, :], scale=1.0)
            rstd = const.tile([BC, 1], f32, tag=tagp + '_rs', name=tagp + '_rs')
            nc.vector.reciprocal(out=rstd, in_=sd)
            nc.vector.tensor_scalar(out=dst, in0=src, scalar1=mv[:, 0:1], scalar2=rstd,
                                    op0=ALU.subtract, op1=ALU.mult)
            nc.vector.tensor_mul(dst, dst, vsb[gname])
            nc.vector.tensor_add(dst, dst, vsb[bname])

        def build_qblk(qsrc_f32, tagp):
            # block-diag q: qb[32g:32g+32, s, 4s+g, b] = q[b, 128s+32g ...]
            qT = make_T(qsrc_f32, tagp + '_qT')
            qb = const.tile([128, 2, H, BC], KDT, tag=tagp + '_qb', name=tagp + '_qb')
            nc.vector.memset(qb, 0.0)
            for s in range(2):
                for g in range(4):
                    h = 4 * s + g
                    nc.vector.tensor_copy(out=qb[32 * g:32 * (g + 1), s, h, :],
                                          in_=qT[32 * g:32 * (g + 1), s, :])
            return qb

        # ---------- qkv for self-attn ----------
        htT = make_T(ht, 'htT')
        qkv = {}
        for nm, wn, bn in (('q', 'wq_s', 'bq_s'), ('k', 'wk_s', 'bk_s'), ('v', 'wv_s', 'bv_s')):
            ps = linear_psum([htT], wn)
            qkv[nm] = const.tile([BC, D], f32, tag='qkv_' + nm, name='qkv_' + nm)
            nc.vector.tensor_add(qkv[nm], ps, vsb[bn])

        qblk_s = build_qblk(qkv['q'], 'self')

        # new-key (appended k/v) weights, all-batch
        qk = const.tile([BC, D], f32, tag='qk')
        nc.vector.tensor_mul(qk, qkv['q'], qkv['k'])
        s_new = const.tile([BC, H], f32, tag='s_new')
        nc.vector.reduce_sum(out=s_new, in_=qk.rearrange('p (g s) -> p g s', g=H), axis=AX.X)
        w_new = const.tile([BC, H], f32, tag='w_new')
        nc.scalar.activation(out=w_new, in_=s_new, func=FX.Exp, scale=SCALE)

        # ---------- attention inner loop ----------
        # scoresT: sc[key, h] = sum_d K[key, d] * qblk[d, h]   (K^T stationary)
        # V-mix:   at[d, h]   = sum_k V[k, d] * w[k, h]        (V stationary)
        # denom:   dn[h]      = sum_k w[k, h]                  (ones stationary)
        def attention(qblk, n_tiles, KT_dram, V_dram, kt_pool, vt_pool,
                      attT_dst, dn_all, masked, self_extra):
            for b in range(BC):
                kt = kt_pool.tile([128, 2, n_tiles, 128], KDT, tag='kt')
                nc.sync.dma_start(out=kt, in_=KT_dram[b])
                vt = vt_pool.tile([128, n_tiles, 2, 128], VDT, tag='vt')
                nc.sync.dma_start(out=vt, in_=V_dram[b])

                pe_absorb(kt, qblk)
                sc = sc_ps.tile([128, n_tiles, H], f32, tag='scps')
                for t in range(n_tiles):
                    nc.tensor.matmul(sc[:, t, :], kt[:, 0, t, :], qblk[:, 0, :, b],
                                     start=True, stop=False, skip_group_check=True)
                    nc.tensor.matmul(sc[:, t, :], kt[:, 1, t, :], qblk[:, 1, :, b],
                                     start=False, stop=True, skip_group_check=True)

                wt = wsb_p.tile([128, n_tiles, H], VDT, tag='wt')
                if masked:
                    wr = wsb_p.tile([128, n_tiles, H], VDT, tag='wr')
                    nc.scalar.activation(out=wr, in_=sc, func=FX.Exp, scale=SCALE)
                    nc.vector.tensor_tensor(
                        out=wt, in0=wr,
                        in1=notmT[:, 0:n_tiles, b].unsqueeze(2).broadcast_to([128, n_tiles, H]),
                        op=ALU.mult)
                else:
                    nc.scalar.activation(out=wt, in_=sc, func=FX.Exp, scale=SCALE)

                pe_absorb(vt, wt)
                at = at_ps.tile([128, 3 * H], f32, tag='atps')
                for t in range(n_tiles):
                    for c in range(2):
                        nc.tensor.matmul(at[:, H * c:H * (c + 1)], vt[:, t, c, :],
                                         wt[:, t, :], start=(t == 0),
                                         stop=(t == n_tiles - 1), skip_group_check=True)
                    nc.tensor.matmul(at[0:1, 2 * H:3 * H], ones128, wt[:, t, :],
                                     start=(t == 0),
                                     stop=(t == n_tiles - 1 and self_extra is None),
                                     skip_group_check=True)
                if self_extra is not None:
                    # += w_new[b, :] (select row b via identity column)
                    nc.tensor.matmul(at[0:1, 2 * H:3 * H], ident[0:BC, b:b + 1],
                                     self_extra, start=False, stop=True,
                                     skip_group_check=True)
                # select head-diagonal columns: attT[d, b] = at[d, head_of(d)]
                for c in range(2):
                    for g in range(4):
                        h = 4 * c + g
                        nc.vector.tensor_copy(
                            out=attT_dst[32 * g:32 * (g + 1), c, b:b + 1],
                            in_=at[32 * g:32 * (g + 1), H * c + h:H * c + h + 1])
                nc.vector.tensor_copy(out=dn_all[0:1, H * b:H * (b + 1)],
                                      in_=at[0:1, 2 * H:3 * H])

        def inv_scale(attT, dn_all, tagp):
            # attT[:, c, b] *= 1 / dn_all[b, h(d)]
            ivf = const.tile([1, BC * H], f32, tag=tagp + '_ivf', name=tagp + '_ivf')
            nc.vector.reciprocal(out=ivf, in_=dn_all)
            inv_row = const.tile([1, BC * H], f32r, tag=tagp + '_ivr', name=tagp + '_ivr')
            nc.vector.tensor_copy(out=inv_row, in_=ivf)
            rep = rp_ps.tile([128, BC * H], f32, tag='rpps')
            pe_absorb(inv_row)
            nc.tensor.matmul(rep[:, :], onescol, inv_row, start=True, stop=True,
                             skip_group_check=True)
            rep_v = rep.rearrange('p (b h) -> p b h', h=H)
            inv_mat = const.tile([128, 2, BC], f32, tag=tagp + '_ivm', name=tagp + '_ivm')
            for c in range(2):
                for g in range(4):
                    h = 4 * c + g
                    nc.vector.tensor_copy(out=inv_mat[32 * g:32 * (g + 1), c, :],
                                          in_=rep_v[32 * g:32 * (g + 1), :, h])
            nc.vector.tensor_tensor(out=attT, in0=attT, in1=inv_mat, op=ALU.mult)

        # ---------- self attention ----------
        attT_s = const.tile([128, 2, BC], f32r, tag='attT_s')
        dn_all_s = const.tile([1, BC * H], f32, tag='dn_all_s')
        attention(qblk_s, KT_S, dr['KcT'], dr['Vc'], kts_p, vts_p,
                  attT_s, dn_all_s, False, w_new)

        # new-key numerator (unnormalized): nv = v * w_new, added before scaling
        nv = const.tile([BC, D], f32, tag='nv')
        nc.vector.tensor_tensor(out=nv.rearrange('p (g s) -> p g s', g=H),
                                in0=qkv['v'].rearrange('p (g s) -> p g s', g=H),
                                in1=w_new.unsqueeze(2).broadcast_to([BC, H, DH]),
                                op=ALU.mult)
        nvT = make_T(nv, 'nvT')
        nc.vector.tensor_tensor(out=attT_s, in0=attT_s, in1=nvT, op=ALU.add)
        inv_scale(attT_s, dn_all_s, 'sf')

        # h1 = LN1(ht + att_self @ w0_s + b0_s)
        ps = linear_psum([attT_s], 'w0_s')
        h1p = const.tile([BC, D], f32, tag='h1p')
        nc.vector.tensor_add(h1p, ps, vsb['b0_s'])
        nc.vector.tensor_add(h1p, h1p, ht)
        h1 = const.tile([BC, D], f32, tag='h1')
        layernorm(h1, h1p, 'ln1_g', 'ln1_b', 'ln1')

        # ---------- cross attention ----------
        h1T = make_T(h1, 'h1T')
        psq = linear_psum([h1T], 'wq_a')
        qa = const.tile([BC, D], f32, tag='qa')
        nc.vector.tensor_add(qa, psq, vsb['bq_a'])
        qblk_a = build_qblk(qa, 'cross')

        attT_a = const.tile([128, 2, BC], f32r, tag='attT_a')
        dn_all_a = const.tile([1, BC * H], f32, tag='dn_all_a')
        attention(qblk_a, KT_A, dr['KaT'], dr['Va'], kta_p, vta_p,
                  attT_a, dn_all_a, True, None)
        inv_scale(attT_a, dn_all_a, 'cr')

        # h2 = LN2(h1 + att_cross @ w0_a + b0_a)
        ps2 = linear_psum([attT_a], 'w0_a')
        h2p = const.tile([BC, D], f32, tag='h2p')
        nc.vector.tensor_add(h2p, ps2, vsb['b0_a'])
        nc.vector.tensor_add(h2p, h2p, h1)
        h2 = const.tile([BC, D], f32, tag='h2')
        layernorm(h2, h2p, 'ln2_g', 'ln2_b', 'ln2')

        # ---------- MLP ----------
        h2T = make_T(h2, 'h2T')
        psm = linear_psum([h2T], 'w1')
        m1 = const.tile([BC, D], f32, tag='m1')
        nc.vector.tensor_add(m1, psm, vsb['b1'])
        m1r = const.tile([BC, D], f32, tag='m1r')
        nc.scalar.activation(out=m1r, in_=m1, func=FX.Relu, scale=1.0)
        pe_absorb(m1r)
        m1T = make_T(m1r, 'm1T')
        psm2 = linear_psum([m1T], 'w2')
        h3p = const.tile([BC, D], f32, tag='h3p')
        nc.vector.tensor_add(h3p, psm2, vsb['b2'])
        nc.vector.tensor_add(h3p, h3p, h2)
        outt = const.tile([BC, D], f32, tag='outt')
        layernorm(outt, h3p, 'ln3_g', 'ln3_b', 'ln3')
        nc.sync.dma_start(out=out_dram[:, :], in_=outt)


_CACHE = {}


def _get_nc():
    if 'nc' not in _CACHE:
        _CACHE['nc'] = _build()
    return _CACHE['nc']


def _kT_tiles(arr, kt):
    # [BC, n, d] -> [BC, p(d%128), s(d//128), t, j(n%128)]
    bc = arr.shape[0]
    return np.ascontiguousarray(
        arr.astype(KDT_NP).reshape(bc, kt, 128, 2, 128).transpose(0, 4, 3, 1, 2))


def _v_tiles(arr, kt):
    # [BC, n, d] -> [BC, p(n%128), t, c(d//128), j(d%128)]
    bc = arr.shape[0]
    return np.ascontiguousarray(
        arr.astype(VDT_NP).reshape(bc, kt, 128, 2, 128).transpose(0, 2, 1, 3, 4))


def _make_in_maps(inputs):
    np_in = {k: np.asarray(v) for k, v in inputs.items()}
    ident = np.eye(128, dtype=np.float32)
    in_maps = []
    for c in range(NCORES):
        sl = slice(c * BC, (c + 1) * BC)
        m = np_in['mask'][sl].astype(np.float32)          # [BC, NA], True = masked
        notm = (1.0 - m).reshape(BC, KT_A, 128).transpose(2, 1, 0).copy()
        im = {
            'h_t': np.ascontiguousarray(np_in['h_t'][sl]),
            'KaT': _kT_tiles(np_in['K_att'][sl], KT_A),
            'Va': _v_tiles(np_in['V_att'][sl], KT_A),
            'KcT': _kT_tiles(np_in['K_cache'][sl], KT_S),
            'Vc': _v_tiles(np_in['V_cache'][sl], KT_S),
            'notmT': notm,
            'ident': ident,
            'onesrow': np.ones((1, 128), dtype=np.float32),
        }
        for n in WNAMES + BNAMES + LNAMES:
            im[n] = np.ascontiguousarray(np_in[n])
        in_maps.append(im)
    return in_maps


def run_on_device(inputs):
    nc = _get_nc()
    in_maps = _make_in_maps(inputs)
    res = bass_utils.run_bass_kernel_spmd(nc, in_maps, core_ids=list(range(NCORES)),
                                          trace=False)
    outs = [res.results[c]['out'] for c in range(NCORES)]
    return np.concatenate(outs, axis=0).astype(np.float32)


def kernel(**inputs):
    return run_on_device(inputs)


# revision 8
# speedup vs baseline: 1.3475x; 1.2932x over previous
import sys
if '/opt/trn_rl_repo' not in sys.path:
    sys.path.insert(0, '/opt/trn_rl_repo')
import numpy as np

import concourse.bass as bass
import concourse.bacc as bacc
import concourse.tile as tile
from concourse import mybir
from concourse import bass_utils

f32 = mybir.dt.float32
f32r = mybir.dt.float32r
bf16 = mybir.dt.bfloat16
FX = mybir.ActivationFunctionType
ALU = mybir.AluOpType
AX = mybir.AxisListType

B, D, H, DH = 256, 256, 8, 32
NCORES = 8
BC = B // NCORES          # 32 batches per core
LC = 1024                 # self-attn KV cache length
NA = 2048                 # cross-attn key count
KT_S = LC // 128          # 8 key tiles (self)
KT_A = NA // 128          # 16 key tiles (cross)
SCALE = 1.0 / float(np.sqrt(DH))
EPS = 1e-5

KDT = mybir.dt.float8e4   # dtype for K^T tiles + q blocks (scores path)
VDT = mybir.dt.float8e4   # dtype for V tiles + softmax weights (mix path)
KDT_NP = mybir.dt.np(KDT)
VDT_NP = mybir.dt.np(VDT)

WNAMES = ['wq_s', 'wk_s', 'wv_s', 'w0_s', 'wq_a', 'w0_a', 'w1', 'w2']
BNAMES = ['bq_s', 'bk_s', 'bv_s', 'b0_s', 'bq_a', 'b0_a', 'b1', 'b2']
LNAMES = ['ln1_g', 'ln1_b', 'ln2_g', 'ln2_b', 'ln3_g', 'ln3_b']


def _build():
    nc = bacc.Bacc()
    dr = {}
    dr['h_t'] = nc.dram_tensor('h_t', [BC, 1, D], f32, kind='ExternalInput')
    # K^T tiles: [b, p(d%128), s(d//128), t, j(key%128)]
    dr['KaT'] = nc.dram_tensor('KaT', [BC, 128, 2, KT_A, 128], KDT, kind='ExternalInput')
    dr['KcT'] = nc.dram_tensor('KcT', [BC, 128, 2, KT_S, 128], KDT, kind='ExternalInput')
    # V tiles: [b, p(key%128), t, c(d//128), j(d%128)]
    dr['Va'] = nc.dram_tensor('Va', [BC, 128, KT_A, 2, 128], VDT, kind='ExternalInput')
    dr['Vc'] = nc.dram_tensor('Vc', [BC, 128, KT_S, 2, 128], VDT, kind='ExternalInput')
    dr['notmT'] = nc.dram_tensor('notmT', [128, KT_A, BC], f32, kind='ExternalInput')
    dr['ident'] = nc.dram_tensor('ident', [128, 128], f32, kind='ExternalInput')
    dr['onesrow'] = nc.dram_tensor('onesrow', [1, 128], f32r, kind='ExternalInput')
    for n in WNAMES:
        dr[n] = nc.dram_tensor(n, [D, D], f32r, kind='ExternalInput')
    for n in BNAMES + LNAMES:
        dr[n] = nc.dram_tensor(n, [D], f32, kind='ExternalInput')
    out = nc.dram_tensor('out', [BC, D], f32, kind='ExternalOutput')

    with tile.TileContext(nc) as tc:
        _emit(nc, tc, dr, out)
    nc.compile()
    return nc


def _emit(nc, tc, dr, out_dram):
    import contextlib
    ctx = contextlib.ExitStack()
    with ctx:
        const = ctx.enter_context(tc.tile_pool(name='const', bufs=1))
        kta_p = ctx.enter_context(tc.tile_pool(name='kta', bufs=3))
        vta_p = ctx.enter_context(tc.tile_pool(name='vta', bufs=3))
        kts_p = ctx.enter_context(tc.tile_pool(name='kts', bufs=3))
        vts_p = ctx.enter_context(tc.tile_pool(name='vts', bufs=3))
        wsb_p = ctx.enter_context(tc.tile_pool(name='wsb', bufs=3))
        sc_ps = ctx.enter_context(tc.tile_pool(name='scps', bufs=2, space='PSUM'))
        at_ps = ctx.enter_context(tc.tile_pool(name='atps', bufs=2, space='PSUM'))
        tr_ps = ctx.enter_context(tc.tile_pool(name='trps', bufs=1, space='PSUM'))
        rp_ps = ctx.enter_context(tc.tile_pool(name='rpps', bufs=1, space='PSUM'))
        ln_ps = ctx.enter_context(tc.tile_pool(name='lnps', bufs=1, space='PSUM'))
        gb_ps = ctx.enter_context(tc.tile_pool(name='gbps', bufs=1, space='PSUM'))

        garb = gb_ps.tile([1, 1], f32, tag='garb')

        def pe_absorb(*aps):
            # PE matmul (self-loading weights) can carry only ONE sem wait in
            # its LW slot. Before a matmul whose deps span several producers,
            # emit 1x1 self-matmuls so the PE observes those sems here.
            for a in aps:
                if a is None:
                    continue
                e = a[tuple(slice(0, 1) for _ in range(len(a.shape)))]
                if e.dtype == f32r:
                    e = e.bitcast(f32)
                nc.tensor.matmul(garb[:, :], e, e, start=True, stop=True,
                                 skip_group_check=True)

        # ---------- persistent loads / consts ----------
        ident = const.tile([128, 128], f32, tag='ident')
        nc.sync.dma_start(out=ident, in_=dr['ident'][:, :])
        pe_absorb(ident)
        epst = const.tile([BC, 1], f32, tag='epst')
        nc.vector.memset(epst, EPS)
        ones128 = const.tile([128, 1], VDT, tag='ones128')
        nc.vector.memset(ones128, 1.0)
        onescol = const.tile([1, 128], f32r, tag='onescol')
        nc.sync.dma_start(out=onescol, in_=dr['onesrow'][:, :])

        wsb = {}
        for n in WNAMES:
            wsb[n] = const.tile([128, 2, D], f32r, tag='w_' + n, name='w_' + n)
            nc.sync.dma_start(out=wsb[n], in_=dr[n][:, :].rearrange('(t p) j -> p t j', p=128))
        vsb = {}
        for n in BNAMES + LNAMES:
            vsb[n] = const.tile([BC, D], f32, tag='v_' + n, name='v_' + n)
            nc.gpsimd.dma_start(out=vsb[n], in_=dr[n][:].unsqueeze(0).to_broadcast([BC, D]))

        notmT = const.tile([128, KT_A, BC], f32, tag='notmT')
        nc.sync.dma_start(out=notmT, in_=dr['notmT'][:, :, :])

        ht = const.tile([BC, D], f32, tag='ht')
        nc.sync.dma_start(out=ht, in_=dr['h_t'][:, 0, :])
        pe_absorb(ht)

        # ---------- helpers ----------
        def transpose_128(dst, src, cols):
            rows = src.shape[0]
            ps = tr_ps.tile([128, 128], f32, tag='trps')
            nc.tensor.transpose(ps[0:cols, 0:rows], src, ident[0:rows, 0:rows])
            nc.vector.tensor_copy(out=dst, in_=ps[0:cols, 0:rows])

        def make_T(src_f32, tagname):
            dstT = const.tile([128, 2, BC], f32r, tag=tagname, name=tagname)
            for t in range(2):
                transpose_128(dstT[:, t, :], src_f32[:, 128 * t:128 * (t + 1)], 128)
            return dstT

        def linear_psum(srcT_list, wname):
            ps = ln_ps.tile([BC, D], f32, tag='lnps')
            pe_absorb(wsb[wname])
            n_mm = 2 * len(srcT_list)
            i = 0
            for srcT in srcT_list:
                for t in range(2):
                    nc.tensor.matmul(ps[:, :], srcT[:, t, :], wsb[wname][:, t, :],
                                     start=(i == 0), stop=(i == n_mm - 1))
                    i += 1
            return ps

        def layernorm(dst, src, gname, bname, tagp):
            stats = const.tile([BC, 6], f32, tag=tagp + '_st', name=tagp + '_st')
            nc.vector.bn_stats(out=stats, in_=src)
            mv = const.tile([BC, 2], f32, tag=tagp + '_mv', name=tagp + '_mv')
            nc.vector.bn_aggr(out=mv, in_=stats)
            sd = const.tile([BC, 1], f32, tag=tagp + '_sd', name=tagp + '_sd')
            nc.scalar.activation(out=sd, in_=mv[:, 1:2], func=FX.Sqrt,
                                 bias=epst[:, :], scale=1.0)
            rstd = const.tile([BC, 1], f32, tag=tagp + '_rs', name=tagp + '_rs')
            nc.vector.reciprocal(out=rstd, in_=sd)
            nc.vector.tensor_scalar(out=dst, in0=src, scalar1=mv[:, 0:1], scalar2=rstd,
                                    op0=ALU.subtract, op1=ALU.mult)
            nc.vector.tensor_mul(dst, dst, vsb[gname])
            nc.vector.tensor_add(dst, dst, vsb[bname])

        def build_qblk(qsrc_f32, tagp):
            # block-diag q: qb[32g:32g+32, s, 4s+g, b] = q[b, 128s+32g ...]
            qT = make_T(qsrc_f32, tagp + '_qT')
            qb = const.tile([128, 2, H, BC], KDT, tag=tagp + '_qb', name=tagp + '_qb')
            nc.vector.memset(qb, 0.0)
            for s in range(2):
                for g in range(4):
                    h = 4 * s + g
                    nc.vector.tensor_copy(out=qb[32 * g:32 * (g + 1), s, h, :],
                                          in_=qT[32 * g:32 * (g + 1), s, :])
            return qb

        # ---------- qkv for self-attn ----------
        htT = make_T(ht, 'htT')
        qkv = {}
        for nm, wn, bn in (('q', 'wq_s', 'bq_s'), ('k', 'wk_s', 'bk_s'), ('v', 'wv_s', 'bv_s')):
            ps = linear_psum([htT], wn)
            qkv[nm] = const.tile([BC, D], f32, tag='qkv_' + nm, name='qkv_' + nm)
            nc.vector.tensor_add(qkv[nm], ps, vsb[bn])

        qblk_s = build_qblk(qkv['q'], 'self')

        # new-key (appended k/v) weights, all-batch
        qk = const.tile([BC, D], f32, tag='qk')
        nc.vector.tensor_mul(qk, qkv['q'], qkv['k'])
        s_new = const.tile([BC, H], f32, tag='s_new')
        nc.vector.reduce_sum(out=s_new, in_=qk.rearrange('p (g s) -> p g s', g=H), axis=AX.X)
        w_new = const.tile([BC, H], f32, tag='w_new')
        nc.scalar.activation(out=w_new, in_=s_new, func=FX.Exp, scale=SCALE)

        # ---------- attention inner loop ----------
        # scoresT: sc[key, h] = sum_d K[key, d] * qblk[d, h]   (K^T stationary)
        # V-mix:   at[d, h]   = sum_k V[k, d] * w[k, h]        (V stationary)
        # denom:   dn[h]      = sum_k w[k, h]                  (ones stationary)
        def attention(qblk, n_tiles, KT_dram, V_dram, kt_pool, vt_pool,
                      attT_dst, dn_all, masked, self_extra):
            for b in range(BC):
                kt = kt_pool.tile([128, 2, n_tiles, 128], KDT, tag='kt')
                nc.sync.dma_start(out=kt, in_=KT_dram[b])
                vt = vt_pool.tile([128, n_tiles, 2, 128], VDT, tag='vt')
                nc.sync.dma_start(out=vt, in_=V_dram[b])

                pe_absorb(kt, qblk)
                sc = sc_ps.tile([128, n_tiles, H], f32, tag='scps')
                for t in range(n_tiles):
                    nc.tensor.matmul(sc[:, t, :], kt[:, 0, t, :], qblk[:, 0, :, b],
                                     start=True, stop=False, skip_group_check=True)
                    nc.tensor.matmul(sc[:, t, :], kt[:, 1, t, :], qblk[:, 1, :, b],
                                     start=False, stop=True, skip_group_check=True)

                wt = wsb_p.tile([128, n_tiles, H], VDT, tag='wt')
                if masked:
                    wr = wsb_p.tile([128, n_tiles, H], VDT, tag='wr')
                    nc.scalar.activation(out=wr, in_=sc, func=FX.Exp, scale=SCALE)
                    nc.vector.tensor_tensor(
                        out=wt, in0=wr,
                        in1=notmT[:, 0:n_tiles, b].unsqueeze(2).broadcast_to([128, n_tiles, H]),
                        op=ALU.mult)
                else:
                    nc.scalar.activation(out=wt, in_=sc, func=FX.Exp, scale=SCALE)

                pe_absorb(vt, wt)
                at = at_ps.tile([128, 3 * H], f32, tag='atps')
                for t in range(n_tiles):
                    for c in range(2):
                        nc.tensor.matmul(at[:, H * c:H * (c + 1)], vt[:, t, c, :],
                                         wt[:, t, :], start=(t == 0),
                                         stop=(t == n_tiles - 1), skip_group_check=True)
                    nc.tensor.matmul(at[0:1, 2 * H:3 * H], ones128, wt[:, t, :],
                                     start=(t == 0),
                                     stop=(t == n_tiles - 1 and self_extra is None),
                                     skip_group_check=True)
                if self_extra is not None:
                    # += w_new[b, :] (select row b via identity column)
                    nc.tensor.matmul(at[0:1, 2 * H:3 * H], ident[0:BC, b:b + 1],
                                     self_extra, start=False, stop=True,
                                     skip_group_check=True)
                # select head-diagonal columns: attT[d, b] = at[d, head_of(d)]
                for c in range(2):
                    for g in range(4):
                        h = 4 * c + g
                        nc.vector.tensor_copy(
                            out=attT_dst[32 * g:32 * (g + 1), c, b:b + 1],
                            in_=at[32 * g:32 * (g + 1), H * c + h:H * c + h + 1])
                nc.vector.tensor_copy(out=dn_all[0:1, H * b:H * (b + 1)],
                                      in_=at[0:1, 2 * H:3 * H])

        def inv_scale(attT, dn_all, tagp):
            # attT[:, c, b] *= 1 / dn_all[b, h(d)]
            ivf = const.tile([1, BC * H], f32, tag=tagp + '_ivf', name=tagp + '_ivf')
            nc.vector.reciprocal(out=ivf, in_=dn_all)
            inv_row = const.tile([1, BC * H], f32r, tag=tagp + '_ivr', name=tagp + '_ivr')
            nc.vector.tensor_copy(out=inv_row, in_=ivf)
            rep = rp_ps.tile([128, BC * H], f32, tag='rpps')
            pe_absorb(inv_row)
            nc.tensor.matmul(rep[:, :], onescol, inv_row, start=True, stop=True,
                             skip_group_check=True)
            rep_v = rep.rearrange('p (b h) -> p b h', h=H)
            inv_mat = const.tile([128, 2, BC], f32, tag=tagp + '_ivm', name=tagp + '_ivm')
            for c in range(2):
                for g in range(4):
                    h = 4 * c + g
                    nc.vector.tensor_copy(out=inv_mat[32 * g:32 * (g + 1), c, :],
                                          in_=rep_v[32 * g:32 * (g + 1), :, h])
            nc.vector.tensor_tensor(out=attT, in0=attT, in1=inv_mat, op=ALU.mult)

        # ---------- self attention ----------
        attT_s = const.tile([128, 2, BC], f32r, tag='attT_s')
        dn_all_s = const.tile([1, BC * H], f32, tag='dn_all_s')
        attention(qblk_s, KT_S, dr['KcT'], dr['Vc'], kts_p, vts_p,
                  attT_s, dn_all_s, False, w_new)

        # new-key numerator (unnormalized): nv = v * w_new, added before scaling
        nv = const.tile([BC, D], f32, tag='nv')
        nc.vector.tensor_tensor(out=nv.rearrange('p (g s) -> p g s', g=H),
                                in0=qkv['v'].rearrange('p (g s) -> p g s', g=H),
                                in1=w_new.unsqueeze(2).broadcast_to([BC, H, DH]),
                                op=ALU.mult)
        nvT = make_T(nv, 'nvT')
        nc.vector.tensor_tensor(out=attT_s, in0=attT_s, in1=nvT, op=ALU.add)
        inv_scale(attT_s, dn_all_s, 'sf')

        # h1 = LN1(ht + att_self @ w0_s + b0_s)
        ps = linear_psum([attT_s], 'w0_s')
        h1p = const.tile([BC, D], f32, tag='h1p')
        nc.vector.tensor_add(h1p, ps, vsb['b0_s'])
        nc.vector.tensor_add(h1p, h1p, ht)
        h1 = const.tile([BC, D], f32, tag='h1')
        layernorm(h1, h1p, 'ln1_g', 'ln1_b', 'ln1')

        # ---------- cross attention ----------
        h1T = make_T(h1, 'h1T')
        psq = linear_psum([h1T], 'wq_a')
        qa = const.tile([BC, D], f32, tag='qa')
        nc.vector.tensor_add(qa, psq, vsb['bq_a'])
        qblk_a = build_qblk(qa, 'cross')

        attT_a = const.tile([128, 2, BC], f32r, tag='attT_a')
        dn_all_a = const.tile([1, BC * H], f32, tag='dn_all_a')
        attention(qblk_a, KT_A, dr['KaT'], dr['Va'], kta_p, vta_p,
                  attT_a, dn_all_a, True, None)
        inv_scale(attT_a, dn_all_a, 'cr')

        # h2 = LN2(h1 + att_cross @ w0_a + b0_a)
        ps2 = linear_psum([attT_a], 'w0_a')
        h2p = const.tile([BC, D], f32, tag='h2p')
        nc.vector.tensor_add(h2p, ps2, vsb['b0_a'])
        nc.vector.tensor_add(h2p, h2p, h1)
        h2 = const.tile([BC, D], f32, tag='h2')
        layernorm(h2, h2p, 'ln2_g', 'ln2_b', 'ln2')

        # ---------- MLP ----------
        h2T = make_T(h2, 'h2T')
        psm = linear_psum([h2T], 'w1')
        m1 = const.tile([BC, D], f32, tag='m1')
        nc.vector.tensor_add(m1, psm, vsb['b1'])
        m1r = const.tile([BC, D], f32, tag='m1r')
        nc.scalar.activation(out=m1r, in_=m1, func=FX.Relu, scale=1.0)
        pe_absorb(m1r)
        m1T = make_T(m1r, 'm1T')
        psm2 = linear_psum([m1T], 'w2')
        h3p = const.tile([BC, D], f32, tag='h3p')
        nc.vector.tensor_add(h3p, psm2, vsb['b2'])
        nc.vector.tensor_add(h3p, h3p, h2)
        outt = const.tile([BC, D], f32, tag='outt')
        layernorm(outt, h3p, 'ln3_g', 'ln3_b', 'ln3')
        nc.sync.dma_start(out=out_dram[:, :], in_=outt)


_CACHE = {}


def _get_nc():
    if 'nc' not in _CACHE:
        _CACHE['nc'] = _build()
    return _CACHE['nc']


def _kT_tiles(arr, kt):
    # [BC, n, d] -> [BC, p(d%128), s(d//128), t, j(n%128)]
    bc = arr.shape[0]
    return np.ascontiguousarray(
        arr.astype(KDT_NP).reshape(bc, kt, 128, 2, 128).transpose(0, 4, 3, 1, 2))


def _v_tiles(arr, kt):
    # [BC, n, d] -> [BC, p(n%128), t, c(d//128), j(d%128)]
    bc = arr.shape[0]
    return np.ascontiguousarray(
        arr.astype(VDT_NP).reshape(bc, kt, 128, 2, 128).transpose(0, 2, 1, 3, 4))


def _make_in_maps(inputs):
    np_in = {k: np.asarray(v) for k, v in inputs.items()}
    ident = np.eye(128, dtype=np.float32)
    in_maps = []
    for c in range(NCORES):
        sl = slice(c * BC, (c + 1) * BC)
        m = np_in['mask'][sl].astype(np.float32)          # [BC, NA], True = masked
        notm = (1.0 - m).reshape(BC, KT_A, 128).transpose(2, 1, 0).copy()
        im = {
            'h_t': np.ascontiguousarray(np_in['h_t'][sl]),
            'KaT': _kT_tiles(np_in['K_att'][sl], KT_A),
            'Va': _v_tiles(np_in['V_att'][sl], KT_A),
            'KcT': _kT_tiles(np_in['K_cache'][sl], KT_S),
            'Vc': _v_tiles(np_in['V_cache'][sl], KT_S),
            'notmT': notm,
            'ident': ident,
            'onesrow': np.ones((1, 128), dtype=np.float32),
        }
        for n in WNAMES + BNAMES + LNAMES:
            im[n] = np.ascontiguousarray(np_in[n])
        in_maps.append(im)
    return in_maps


def run_on_device(inputs):
    nc = _get_nc()
    in_maps = _make_in_maps(inputs)
    res = bass_utils.run_bass_kernel_spmd(nc, in_maps, core_ids=list(range(NCORES)),
                                          trace=False)
    outs = [res.results[c]['out'] for c in range(NCORES)]
    return np.concatenate(outs, axis=0).astype(np.float32)


def kernel(**inputs):
    return run_on_device(inputs)
